# revision 1
# baseline (speedup 1.0000x reference)
"""DeepAR Trainium2 Bass kernel (fp8 DoubleRow edition).

Strategy (hardcoded from spec nn_DeepAR_90374701843258):
  B=32, LIN=96, LOUT=24, N=256, E=32, H=64, T-1=119 steps, 8 cores.
  Data-parallel over B: 4 batch rows per core -> per-core batch BN=1024.

  Layout: folded tiles: partition p<64 = H-unit p of batch half 0
  (bn 0:512), p>=64 = H-unit p-64 of half 1. Free dim = within-half
  batch column (512 wide).

  PE: all gate matmuls are fp8(e4m3) DoubleRow: one instruction
  contracts TWO 128-row groups (0.5 cyc/row). Layer0 per gate:
  groups (Whh0-fold x h0(t-1), [w_eff;b_eff rows] x [x;ones rows]).
  Layer1 per gate: bias DoubleRow (hi + residual*16 against constant
  rows) then (Whh1-fold x h1(t-1), Wih1-fold x h0(t)).

  Hidden history big-tile HH (fp8): per step t two 512-wide slots,
  A(t) = x(t) rows {0,64} + ones rows {1,2}, later overwritten by
  h1(t-1); B(t) = h0(t). Consecutive slots give contiguous [128,2,512]
  DoubleRow rhs views for both layers; h1 history stays resident so
  the mu/sigma head matmuls run as a post-loop pass (no in-loop PSUM
  pressure; PSUM = 4 banks layer0 + 4 banks layer1 exactly).

  Activations: all-tanh (i,f,o weights pre-halved; sigmoid recovered
  as 0.5*t+0.5 folded into the cell algebra; h stored as 2h with all
  h-consuming weights pre-halved) -> ONE ACT call per layer per step.
  Cell: u=(ti+1)*tg, v=(tf+1)*c, s=u+v, c'=0.5s, tc=tanh(0.5s) via
  ACT scale, h2=(to+1)*tc written straight to HH as fp8.
"""

import numpy as np

B, LIN, LOUT, N, E, H = 32, 96, 24, 256, 32, 64
T = LIN + LOUT
TS = T - 1            # 119
NCORES = 8
BL = B // NCORES      # 4
BN = BL * N           # 1024
HALF = 512
NCHUNK = BN // 128    # 8
SLOT = 1024           # A+B slot pair width per step in HH
HHW = (TS + 2) * SLOT  # HH free width

_cache = {}


def _pack_weights(inp):
    """Host-side weight prep (tiny arrays)."""
    import ml_dtypes
    fp8 = ml_dtypes.float8_e4m3fn
    f32 = np.float32

    Wih0, Whh0 = inp["Wih0"].astype(f32), inp["Whh0"].astype(f32)
    Wih1, Whh1 = inp["Wih1"].astype(f32), inp["Whh1"].astype(f32)
    w_eff = (Wih0 @ inp["embed_W"].astype(f32))[:, 0]
    b_eff = (Wih0 @ inp["embed_b"].astype(f32) + inp["bih0"] + inp["bhh0"]).astype(f32)
    b1 = (inp["bih1"] + inp["bhh1"]).astype(f32)

    # per-gate scale: tanh-trick halves i,f,o args; g full
    gs = np.ones(4 * H, f32)
    gs[0:H] = 0.5
    gs[H:2 * H] = 0.5
    gs[3 * H:] = 0.5

    def q8(x):
        return np.asarray(x, f32).astype(fp8)

    def fold(Wm):
        # Wm [64 out-units, 64 in-units] -> block-diag [128, 128]
        out = np.zeros((128, 128), f32)
        out[0:64, 0:64] = Wm.T
        out[64:128, 64:128] = Wm.T
        return out

    # h stored as 2h -> h-consuming weights * 0.5
    WD0 = np.zeros((128, 4, 2, 128), f32)
    WD1 = np.zeros((128, 4, 2, 128), f32)
    BD1 = np.zeros((128, 4, 2, 128), f32)
    W1S = np.zeros((128, 4, 128), f32)
    for X in range(4):
        g = gs[X * H]
        WD0[:, X, 0, :] = fold(Whh0[X * H:(X + 1) * H] * g * 0.5)
        we = w_eff[X * H:(X + 1) * H] * g
        be = b_eff[X * H:(X + 1) * H] * g
        be_hi = q8(be).astype(f32)
        be_res = (be - be_hi) * 16.0
        WD0[0, X, 1, 0:64] = we
        WD0[64, X, 1, 64:128] = we
        WD0[1, X, 1, 0:64] = be_hi
        WD0[1, X, 1, 64:128] = be_hi
        WD0[2, X, 1, 0:64] = be_res
        WD0[2, X, 1, 64:128] = be_res

        WD1[:, X, 0, :] = fold(Whh1[X * H:(X + 1) * H] * g * 0.5)
        WD1[:, X, 1, :] = fold(Wih1[X * H:(X + 1) * H] * g * 0.5)
        W1S[:, X, :] = WD1[:, X, 1, :]

        bb = b1[X * H:(X + 1) * H] * g
        bb_hi = q8(bb).astype(f32)
        bb_res = (bb - bb_hi) * 16.0
        BD1[1, X, 0, 0:64] = bb_hi
        BD1[1, X, 0, 64:128] = bb_hi
        BD1[2, X, 1, 0:64] = bb_res
        BD1[2, X, 1, 64:128] = bb_res

    ONES2 = np.zeros((128, 1024), f32)
    ONES2[1, 0:512] = 1.0
    ONES2[2, 512:1024] = 0.0625

    # x-slot fill plane: row1 = 1.0 (bias), row2 = 0.0625 (bias residual),
    # all other non-x rows zero (kills SBUF garbage under zero lhsT rows)
    TS_ = 119
    XPLANE = np.zeros((128, TS_ * 512), f32)
    XPLANE[1, :] = 1.0
    XPLANE[2, :] = 0.0625

    HD2 = np.zeros((128, 4), f32)
    HD2[0:64, 0] = inp["mu_W"].astype(f32)[0] * 0.5
    HD2[0:64, 1] = inp["sigma_W"].astype(f32)[0] * 0.5
    HD2[64:128, 2] = inp["mu_W"].astype(f32)[0] * 0.5
    HD2[64:128, 3] = inp["sigma_W"].astype(f32)[0] * 0.5

    return {
        "WD0": q8(WD0.reshape(128, 1024)),
        "WD1": q8(WD1.reshape(128, 1024)),
        "BD1": q8(BD1.reshape(128, 1024)),
        "W1S": q8(W1S.reshape(128, 512)),
        "ONES2": q8(ONES2),
        "XPLANE": q8(XPLANE),
        "HD2": q8(HD2),
        "IDB": np.eye(128, dtype=f32).astype(ml_dtypes.bfloat16),
        "IDF": np.eye(128, dtype=f32),
        "mu_b": float(inp["mu_b"][0]), "sigma_b": float(inp["sigma_b"][0]),
    }


def _build(mu_b, sigma_b):
    """Build the per-core bass program (SPMD: identical on all cores)."""
    from contextlib import ExitStack
    import concourse.mybir as mybir
    import concourse.tile as tile
    from concourse import bacc

    dt = mybir.dt
    AF = mybir.ActivationFunctionType
    OP = mybir.AluOpType
    DR = mybir.MatmulPerfMode.DoubleRow

    nc = bacc.Bacc()

    # ---- I/O ----------------------------------------------------------
    hist = nc.declare_dram_parameter("hist", [BL, LIN, N], dt.float32, isOutput=False)
    fut = nc.declare_dram_parameter("fut", [BL, LOUT, N], dt.float32, isOutput=False)
    hmask = nc.declare_dram_parameter("hmask", [BL, LIN, N], dt.float32, isOutput=False)
    fmask = nc.declare_dram_parameter("fmask", [BL, LOUT, N], dt.float32, isOutput=False)
    epsin = nc.declare_dram_parameter("epsin", [BL, TS, N], dt.float32, isOutput=False)
    wWD0 = nc.declare_dram_parameter("WD0", [128, 1024], dt.float8e4, isOutput=False)
    wWD1 = nc.declare_dram_parameter("WD1", [128, 1024], dt.float8e4, isOutput=False)
    wBD1 = nc.declare_dram_parameter("BD1", [128, 1024], dt.float8e4, isOutput=False)
    wON2 = nc.declare_dram_parameter("ONES2", [128, 1024], dt.float8e4, isOutput=False)
    wXPL = nc.declare_dram_parameter("XPLANE", [128, TS * HALF], dt.float8e4, isOutput=False)
    wHD2 = nc.declare_dram_parameter("HD2", [128, 4], dt.float8e4, isOutput=False)
    wIDB = nc.declare_dram_parameter("IDB", [128, 128], dt.bfloat16, isOutput=False)
    wIDF = nc.declare_dram_parameter("IDF", [128, 128], dt.float32, isOutput=False)

    o_preds = nc.declare_dram_parameter("preds", [BL, TS, N], dt.float32, isOutput=True)
    o_reals = nc.declare_dram_parameter("reals", [BL, TS, N], dt.float32, isOutput=True)
    o_mus = nc.declare_dram_parameter("musv", [BL, TS, N], dt.float32, isOutput=True)
    o_sigs = nc.declare_dram_parameter("sigmasv", [BL, TS, N], dt.float32, isOutput=True)
    o_mask = nc.declare_dram_parameter("maskv", [BL, TS, N], dt.float32, isOutput=True)

    musig_d = nc.dram_tensor("musig", [4, TS, HALF], dt.float32)
    xs_d = nc.dram_tensor("xsd", [TS, BN], dt.float8e4)

    # HH slot offsets (elements within HH free dim)
    def A_off(t):
        return SLOT + t * SLOT

    def B_off(t):
        return SLOT + t * SLOT + HALF   # B(-1) = 512

    with ExitStack() as ctx:
        tc = ctx.enter_context(tile.TileContext(nc))
        persist = ctx.enter_context(tc.tile_pool(name="persist", bufs=1))
        work = ctx.enter_context(tc.tile_pool(name="work", bufs=2))
        psl0 = ctx.enter_context(tc.tile_pool(name="psl0", bufs=1, space="PSUM"))
        psl1 = ctx.enter_context(tc.tile_pool(name="psl1", bufs=1, space="PSUM"))

        # ---- weights ------------------------------------------------------
        WD0 = persist.tile([128, 1024], dt.float8e4, tag="WD0")
        WD1 = persist.tile([128, 1024], dt.float8e4, tag="WD1")
        BD1 = persist.tile([128, 1024], dt.float8e4, tag="BD1")
        HD2 = persist.tile([128, 4], dt.float8e4, tag="HD2")
        IDB = persist.tile([128, 128], dt.bfloat16, tag="IDB")
        IDF = persist.tile([128, 128], dt.float32, tag="IDF")
        for t_, d_ in [(WD0, wWD0), (WD1, wWD1), (BD1, wBD1),
                       (HD2, wHD2), (IDB, wIDB), (IDF, wIDF)]:
            nc.sync.dma_start(out=t_[:], in_=d_[:])

        WD0v = WD0.rearrange("p (x g c) -> p x g c", x=4, g=2)
        WD1v = WD1.rearrange("p (x g c) -> p x g c", x=4, g=2)
        BD1v = BD1.rearrange("p (x g c) -> p x g c", x=4, g=2)

        # ---- big tiles / state --------------------------------------------
        HH = persist.tile([128, HHW], dt.float8e4, tag="HH")
        ONES2 = persist.tile([128, 1024], dt.float8e4, tag="ONES2")
        nc.sync.dma_start(out=ONES2[:], in_=wON2[:])

        tg0 = persist.tile([128, 2048], dt.bfloat16, tag="tg0")
        tg1 = persist.tile([128, 2048], dt.bfloat16, tag="tg1")
        Sb = [persist.tile([128, 1024], dt.bfloat16, tag=f"S{i}", name=f"S{i}")
              for i in range(2)]
        TCb = [persist.tile([128, 1024], dt.bfloat16, tag=f"TC{i}", name=f"TC{i}")
               for i in range(2)]
        u0 = persist.tile([128, HALF], dt.bfloat16, tag="u0")
        v0 = persist.tile([128, HALF], dt.bfloat16, tag="v0")
        u1 = persist.tile([128, HALF], dt.bfloat16, tag="u1")
        v1 = persist.tile([128, HALF], dt.bfloat16, tag="v1")
        c0t = persist.tile([128, HALF], dt.bfloat16, tag="c0t")
        c1t = persist.tile([128, HALF], dt.bfloat16, tag="c1t")
        nc.vector.memset(c0t, 0.0)
        nc.vector.memset(c1t, 0.0)

        # pad pair [0:1024): (h1(-1)=0, h0(0) copy) for l1's t=0 DoubleRow;
        # B(-1) = h0(-1) = 0 for l0's t=0
        nc.vector.memset(HH[:, 0:HALF], 0.0)
        nc.vector.memset(HH[:, B_off(-1):B_off(-1) + HALF], 0.0)

        def arows(r0, r1):
            # view of HH rows [r0:r1), A-slots t=0..TS-1: [r1-r0, TS, 512]
            return HH[r0:r1, A_off(0):A_off(TS - 1) + HALF + HALF].rearrange(
                "p (t s) -> p t s", s=SLOT)[:, :, 0:HALF]

        # fill non-x rows of every x slot from the host plane (bias consts
        # on rows 1/2, zeros elsewhere so garbage never meets the PE)
        xplv = wXPL.rearrange("p (t s) -> p t s", s=HALF)
        nc.sync.dma_start(out=arows(1, 64), in_=xplv[1:64])
        nc.sync.dma_start(out=arows(65, 128), in_=xplv[65:128])

        full_c, mv_c, stdev_c, istd_c = [], [], [], []

        # ---- pre-pass: stats, normalize, transpose x ----------------------
        xt8 = persist.tile([TS, BN], dt.float8e4, tag="xt8")
        for c in range(NCHUNK):
            b_, n0 = c // 2, (c % 2) * 128
            raw = work.tile([T, 128], dt.float32, tag="raw")
            nc.sync.dma_start(out=raw[0:LIN, :], in_=hist[b_, :, n0:n0 + 128])
            nc.sync.dma_start(out=raw[LIN:T, :], in_=fut[b_, :, n0:n0 + 128])
            fpt = psl1.tile([128, T], dt.float32, tag="l1", name="fpt")
            nc.tensor.transpose(fpt, raw, IDF[0:T, 0:T])
            fc = persist.tile([128, T], dt.float32, tag=f"full{c}", name=f"full{c}")
            nc.vector.tensor_copy(fc, fpt)

            st6 = work.tile([128, 6], dt.float32, tag="st6")
            mv = persist.tile([128, 2], dt.float32, tag=f"mv{c}", name=f"mv{c}")
            nc.vector.bn_stats(out=st6, in_=fc[:, 0:LIN])
            nc.vector.bn_aggr(out=mv, in_=st6)
            veps = work.tile([128, 1], dt.float32, tag="veps")
            nc.vector.tensor_scalar(out=veps, in0=mv[:, 1:2], scalar1=1e-5,
                                    scalar2=None, op0=OP.add)
            y0 = work.tile([128, 1], dt.float32, tag="y0")
            nc.scalar.activation(y0, veps, AF.Sqrt)
            r0 = work.tile([128, 1], dt.float32, tag="r0")
            nc.vector.reciprocal(r0, y0)
            yy = work.tile([128, 1], dt.float32, tag="yy")
            nc.vector.tensor_tensor(out=yy, in0=y0, in1=y0, op=OP.mult)
            e_ = work.tile([128, 1], dt.float32, tag="e_")
            nc.vector.tensor_tensor(out=e_, in0=veps, in1=yy, op=OP.subtract)
            d_ = work.tile([128, 1], dt.float32, tag="d_")
            nc.vector.scalar_tensor_tensor(out=d_, in0=e_, scalar=0.5, in1=r0,
                                           op0=OP.mult, op1=OP.mult)
            sd = persist.tile([128, 1], dt.float32, tag=f"sd{c}", name=f"sd{c}")
            nc.vector.tensor_tensor(out=sd, in0=y0, in1=d_, op=OP.add)
            isd = persist.tile([128, 1], dt.float32, tag=f"isd{c}", name=f"isd{c}")
            nc.vector.reciprocal(isd, sd)
            full_c.append(fc); mv_c.append(mv); stdev_c.append(sd); istd_c.append(isd)

            xn = work.tile([128, TS], dt.bfloat16, tag="xn")
            nc.vector.tensor_scalar(out=xn, in0=fc[:, 0:TS], scalar1=mv[:, 0:1],
                                    scalar2=isd, op0=OP.subtract, op1=OP.mult)
            pt = psl0.tile([TS, 128], dt.bfloat16, tag="l0", name="pt")
            nc.tensor.transpose(pt, xn, IDB)
            nc.vector.tensor_copy(xt8[:, c * 128:(c + 1) * 128], pt)

        # stage xt8 -> DRAM -> HH A-slot rows 0 / 64
        nc.sync.dma_start(out=xs_d[:], in_=xt8[:])
        nc.sync.dma_start(out=arows(0, 1), in_=xs_d[None, :, 0:HALF])
        nc.sync.dma_start(out=arows(64, 65), in_=xs_d[None, :, HALF:BN])

        # ---- rhs view helpers --------------------------------------------
        def rhs2(off):
            # [128, 2, 512] contiguous DoubleRow rhs starting at element off
            return HH[:, off:off + 2 * HALF].rearrange("p (g s) -> p g s", s=HALF)

        def l0_mm(ps, t):
            # groups (h0(t-1) @ B(t-1), x(t) @ A(t)), stride 512
            rv = rhs2(B_off(t - 1))
            for X in range(4):
                nc.tensor.matmul(ps[:, X * HALF:(X + 1) * HALF],
                                 lhsT=WD0v[:, X], rhs=rv,
                                 start=True, stop=True, perf_mode=DR)

        def l1_mm(ps, t):
            ov = ONES2.rearrange("p (g s) -> p g s", s=HALF)
            # groups (h1(t-1) @ A(t), h0(t) @ B(t)), stride 512.  For t=0
            # the pad pair [0:1024) holds (zeros, copy of h0(0)).
            rv = rhs2(A_off(t)) if t >= 1 else rhs2(0)
            for X in range(4):
                nc.tensor.matmul(ps[:, X * HALF:(X + 1) * HALF],
                                 lhsT=BD1v[:, X], rhs=ov,
                                 start=True, stop=False, perf_mode=DR)
                nc.tensor.matmul(ps[:, X * HALF:(X + 1) * HALF],
                                 lhsT=WD1v[:, X], rhs=rv,
                                 start=False, stop=True, perf_mode=DR)

        def cell_uvs(tg, ct, ut, vt, shalf):
            ti = tg[:, 0:HALF]
            tf = tg[:, HALF:2 * HALF]
            tgg = tg[:, 2 * HALF:3 * HALF]
            nc.vector.scalar_tensor_tensor(out=vt, in0=tf, scalar=1.0, in1=ct,
                                           op0=OP.add, op1=OP.mult)
            nc.vector.scalar_tensor_tensor(out=ut, in0=ti, scalar=1.0, in1=tgg,
                                           op0=OP.add, op1=OP.mult)
            nc.vector.tensor_tensor(out=shalf, in0=ut, in1=vt, op=OP.add)

        def cell_fin(tg, ct, shalf, tchalf, hout):
            # c' = 0.5*s ; h2 = (to+1)*tc
            to = tg[:, 3 * HALF:4 * HALF]
            nc.vector.tensor_scalar(out=ct, in0=shalf, scalar1=0.5,
                                    scalar2=None, op0=OP.mult)
            nc.vector.scalar_tensor_tensor(out=hout, in0=to, scalar=1.0,
                                           in1=tchalf, op0=OP.add, op1=OP.mult)

        # ---- main loop ----------------------------------------------------
        for t in range(TS):
            S = Sb[t % 2]
            TC = TCb[t % 2]
            l0ps = psl0.tile([128, 2048], dt.float32, tag="l0", name="l0ps")
            l0_mm(l0ps, t)
            nc.scalar.activation(tg0, l0ps, AF.Tanh)
            cell_uvs(tg0, c0t, u0, v0, S[:, 0:HALF])
            if t >= 1:
                l1ps = psl1.tile([128, 2048], dt.float32, tag="l1", name="l1ps")
                l1_mm(l1ps, t - 1)
                nc.scalar.activation(tg1, l1ps, AF.Tanh)
                cell_uvs(tg1, c1t, u1, v1, S[:, HALF:1024])
                nc.scalar.activation(TC, S, AF.Tanh, scale=0.5)
            else:
                nc.scalar.activation(TC[:, 0:HALF], S[:, 0:HALF], AF.Tanh,
                                     scale=0.5)
            cell_fin(tg0, c0t, S[:, 0:HALF], TC[:, 0:HALF],
                     HH[:, B_off(t):B_off(t) + HALF])
            if t == 0:
                # copy of h0(0) into the pad pair for l1's t=0 DoubleRow
                nc.vector.scalar_tensor_tensor(
                    out=HH[:, HALF:2 * HALF], in0=tg0[:, 3 * HALF:4 * HALF],
                    scalar=1.0, in1=TC[:, 0:HALF], op0=OP.add, op1=OP.mult)
            if t >= 1:
                cell_fin(tg1, c1t, S[:, HALF:1024], TC[:, HALF:1024],
                         HH[:, A_off(t):A_off(t) + HALF])

        # drain layer1 for t = TS-1
        t = TS
        S = Sb[t % 2]
        TC = TCb[t % 2]
        l1ps = psl1.tile([128, 2048], dt.float32, tag="l1", name="l1ps")
        l1_mm(l1ps, t - 1)
        nc.scalar.activation(tg1, l1ps, AF.Tanh)
        cell_uvs(tg1, c1t, u1, v1, S[:, HALF:1024])
        nc.scalar.activation(TC[:, HALF:1024], S[:, HALF:1024], AF.Tanh,
                             scale=0.5)
        cell_fin(tg1, c1t, S[:, HALF:1024], TC[:, HALF:1024],
                 HH[:, A_off(t):A_off(t) + HALF])

        # ---- heads tail: mu/sigma for every step from h1 history ----------
        for k in range(0, TS, 4):
            pool = psl0 if (k // 4) % 2 == 0 else psl1
            tagn = "l0" if (k // 4) % 2 == 0 else "l1"
            hp = pool.tile([128, 2048], dt.float32, tag=tagn, name="hp")
            kk = min(4, TS - k)
            for j in range(kk):
                nc.tensor.matmul(hp[0:4, j * HALF:(j + 1) * HALF],
                                 lhsT=HD2[:, 0:4],
                                 rhs=HH[:, A_off(k + j + 1):A_off(k + j + 1) + HALF],
                                 start=True, stop=True)
            hs = work.tile([4, 2048], dt.float32, tag="hs", bufs=2)
            if (k // 4) % 2 == 0:
                nc.vector.tensor_copy(hs[:, 0:kk * HALF], hp[0:4, 0:kk * HALF])
            else:
                nc.scalar.activation(hs[:, 0:kk * HALF], hp[0:4, 0:kk * HALF],
                                     AF.Copy)
            nc.sync.dma_start(
                out=musig_d[:, k:k + kk, :],
                in_=hs[:, 0:kk * HALF].rearrange("p (s b) -> p s b", b=HALF))

        # ---- post-pass ----------------------------------------------------
        c_sigb = persist.tile([128, 1], dt.float32, tag="c_sigb")
        nc.vector.memset(c_sigb, sigma_b)
        c_neg1 = persist.tile([128, 1], dt.float32, tag="c_neg1")
        nc.vector.memset(c_neg1, -1.0)

        for c in range(NCHUNK):
            b_, n0 = c // 2, (c % 2) * 128
            fc, mv, sd, isd = full_c[c], mv_c[c], stdev_c[c], istd_c[c]
            hh_, nb = c // 4, c % 4

            mu_raw = work.tile([TS, 128], dt.float32, tag="mu_raw")
            nc.sync.dma_start(out=mu_raw,
                              in_=musig_d[2 * hh_, :, nb * 128:(nb + 1) * 128])
            sg_raw = work.tile([TS, 128], dt.float32, tag="sg_raw")
            nc.sync.dma_start(out=sg_raw,
                              in_=musig_d[2 * hh_ + 1, :, nb * 128:(nb + 1) * 128])
            mu_ps = psl0.tile([128, TS], dt.float32, tag="l0", name="mu_ps")
            nc.tensor.transpose(mu_ps, mu_raw, IDF[0:TS, 0:TS])
            sg_ps = psl1.tile([128, TS], dt.float32, tag="l1", name="sg_ps")
            nc.tensor.transpose(sg_ps, sg_raw, IDF[0:TS, 0:TS])
            mu_t = work.tile([128, TS], dt.float32, tag="mu_t")
            nc.vector.tensor_copy(mu_t, mu_ps)
            sg_t = work.tile([128, TS], dt.float32, tag="sg_t")
            nc.vector.tensor_copy(sg_t, sg_ps)

            eps_c = work.tile([128, TS], dt.float32, tag="eps_c")
            nc.sync.dma_start(out=eps_c,
                              in_=epsin[b_, :, n0:n0 + 128].rearrange("t n -> n t"))
            mk = work.tile([128, TS], dt.float32, tag="mk")
            nc.sync.dma_start(out=mk[:, 0:LIN - 1],
                              in_=hmask[b_, 1:LIN, n0:n0 + 128].rearrange("t n -> n t"))
            nc.sync.dma_start(out=mk[:, LIN - 1:TS],
                              in_=fmask[b_, :, n0:n0 + 128].rearrange("t n -> n t"))

            # sigma = softplus(sg + sigma_b) + 1e-6
            ab_ = work.tile([128, TS], dt.float32, tag="ab_")
            nc.scalar.activation(ab_, sg_t, AF.Abs, bias=c_sigb)
            ex_ = work.tile([128, TS], dt.float32, tag="ex_")
            nc.scalar.activation(ex_, ab_, AF.Exp, scale=c_neg1)
            ln_ = work.tile([128, TS], dt.float32, tag="ln_")
            nc.scalar.activation(ln_, ex_, AF.Ln, bias=1.0)
            rl_ = work.tile([128, TS], dt.float32, tag="rl_")
            nc.vector.tensor_scalar(out=rl_, in0=sg_t, scalar1=sigma_b,
                                    scalar2=0.0, op0=OP.add, op1=OP.max)
            sig = work.tile([128, TS], dt.float32, tag="sig")
            nc.vector.scalar_tensor_tensor(out=sig, in0=ln_, scalar=1e-6, in1=rl_,
                                           op0=OP.add, op1=OP.add)

            # preds = ((mu+mu_b) + sigma*eps)*stdev + means, masked
            m1 = work.tile([128, TS], dt.float32, tag="m1")
            nc.vector.tensor_tensor(out=m1, in0=sig, in1=eps_c, op=OP.mult)
            m2 = work.tile([128, TS], dt.float32, tag="m2")
            nc.vector.scalar_tensor_tensor(out=m2, in0=mu_t, scalar=mu_b, in1=m1,
                                           op0=OP.add, op1=OP.add)
            m3 = work.tile([128, TS], dt.float32, tag="m3")
            nc.vector.tensor_scalar(out=m3, in0=m2, scalar1=sd, scalar2=mv[:, 0:1],
                                    op0=OP.mult, op1=OP.add)
            pr = work.tile([128, TS], dt.float32, tag="pr")
            nc.vector.tensor_tensor(out=pr, in0=m3, in1=mk, op=OP.mult)

            rr = work.tile([128, TS], dt.float32, tag="rr")
            nc.vector.tensor_tensor(out=rr, in0=fc[:, 1:T], in1=mk, op=OP.mult)

            u1_ = work.tile([128, TS], dt.float32, tag="u1_")
            nc.vector.tensor_scalar(out=u1_, in0=mu_t, scalar1=mu_b, scalar2=None,
                                    op0=OP.add)
            u2 = work.tile([128, TS], dt.float32, tag="u2")
            nc.vector.tensor_scalar(out=u2, in0=u1_, scalar1=sd, scalar2=mv[:, 0:1],
                                    op0=OP.mult, op1=OP.add)

            v1_ = work.tile([128, TS], dt.float32, tag="v1_")
            nc.vector.tensor_scalar(out=v1_, in0=sig, scalar1=sd, scalar2=mv[:, 0:1],
                                    op0=OP.mult, op1=OP.add)

            for src_t, odram in ((pr, o_preds), (rr, o_reals), (u2, o_mus),
                                 (v1_, o_sigs), (mk, o_mask)):
                tps = psl0.tile([TS, 128], dt.float32, tag="l0", name="tps")
                nc.tensor.transpose(tps, src_t, IDF)
                osb = work.tile([TS, 128], dt.float32, tag="osb", bufs=4)
                nc.vector.tensor_copy(osb, tps)
                nc.sync.dma_start(out=odram[b_, :, n0:n0 + 128], in_=osb)

    nc.finalize()
    return nc


def kernel(**inputs):
    import os
    from concourse.bass_utils import run_bass_kernel_spmd

    f32 = np.float32
    packs = _pack_weights(inputs)

    key = "nc"
    if key not in _cache:
        _cache[key] = _build(packs["mu_b"], packs["sigma_b"])
    nc = _cache[key]

    hist = np.ascontiguousarray(np.asarray(inputs["history_data"], f32)[..., 0])
    fut = np.ascontiguousarray(np.asarray(inputs["future_data"], f32)[..., 0])
    hm = np.ascontiguousarray(np.asarray(inputs["history_mask"], f32))
    fm = np.ascontiguousarray(np.asarray(inputs["future_mask"], f32))
    eps = np.ascontiguousarray(np.asarray(inputs["eps"], f32)[..., 0])

    in_maps = []
    for c in range(NCORES):
        b0, b1 = c * BL, (c + 1) * BL
        m = {
            "hist": hist[b0:b1], "fut": fut[b0:b1],
            "hmask": hm[b0:b1], "fmask": fm[b0:b1], "epsin": eps[b0:b1],
        }
        for k in ("WD0", "WD1", "BD1", "ONES2", "XPLANE", "HD2", "IDB", "IDF"):
            m[k] = packs[k]
        in_maps.append(m)

    kres = run_bass_kernel_spmd(nc, in_maps, list(range(NCORES)),
                                trace=bool(os.environ.get("KERNEL_TRACE")))
    _cache["last"] = kres
    res = kres.results

    def gather(name):
        full = np.concatenate([res[c][name] for c in range(NCORES)], axis=0)
        return full.reshape(B, TS, N, 1).astype(f32)

    return (gather("preds"), gather("reals"), gather("musv"),
            gather("sigmasv"), gather("maskv"))



# revision 4
# speedup vs baseline: 1.1911x; 1.1911x over previous
"""DeepAR Trainium2 Bass kernel (sigmoid-native edition).

Strategy (hardcoded from spec nn_DeepAR_90374701843258):
  B=32, LIN=96, LOUT=24, N=256, E=32, H=64, T-1=119 steps, 8 cores.
  Data-parallel over B: 4 batch rows per core -> per-core batch BN=1024.

  Layout: folded tiles: partition p<64 = H-unit p of batch half 0
  (bn 0:512), p>=64 = H-unit p-64 of half 1. Free dim = within-half
  batch column (512 wide).

  PE: gate matmuls fp8(e4m3) DoubleRow, 4 per layer per step.
  Layer0 per gate: groups (Whh0-fold x h0(t-1), [w_eff;bias rows] x
  [x;ones rows]).  Layer1 per gate: groups (Whh1-fold x h1(t-1),
  Wih1-fold x h0(t)); layer1 bias applied in fp32 via the ACT bias
  operand (per-gate [128,1]) -- no bias matmuls.

  Activations: native Sigmoid for i,f,o and Tanh for g / tanh(c) --
  both functions live in the same HW activation table
  (sigmoid_and_others) so there are zero table swaps in the loop.
  Gate PSUM order (i,f,o,g) so layer0 sigmoids are ONE [128,1536] ACT.
  Cell math is pure TENSOR_TENSOR (2x DVE mode): a=sf*c, b=si*tg,
  c'=a+b, h=so*tanh(c') with h written straight to HH as fp8.

  Hidden history big-tile HH (fp8): per step t two 512-wide slots,
  A(t) = x(t) rows {0,64} + ones rows {1,2} (zeros elsewhere, filled
  by queue-split XPLANE DMAs), later overwritten by h1(t-1); B(t) =
  h0(t).  Consecutive slots give contiguous [128,2,512] DoubleRow rhs
  views for both layers; h1 history stays resident so mu/sigma heads
  run post-loop, partition-packed 8 steps per [32,512] PSUM bank.
"""

import numpy as np

B, LIN, LOUT, N, E, H = 32, 96, 24, 256, 32, 64
T = LIN + LOUT
TS = T - 1            # 119
NCORES = 8
BL = B // NCORES      # 4
BN = BL * N           # 1024
HALF = 512
NCHUNK = BN // 128    # 8
SLOT = 1024           # A+B slot pair width per step in HH
HHW = (TS + 2) * SLOT  # HH free width

_cache = {}


def _pack_weights(inp):
    """Host-side weight prep (tiny arrays)."""
    import ml_dtypes
    fp8 = ml_dtypes.float8_e4m3fn
    f32 = np.float32

    Wih0, Whh0 = inp["Wih0"].astype(f32), inp["Whh0"].astype(f32)
    Wih1, Whh1 = inp["Wih1"].astype(f32), inp["Whh1"].astype(f32)
    w_eff = (Wih0 @ inp["embed_W"].astype(f32))[:, 0]
    b_eff = (Wih0 @ inp["embed_b"].astype(f32) + inp["bih0"] + inp["bhh0"]).astype(f32)
    b1 = (inp["bih1"] + inp["bhh1"]).astype(f32)

    # PSUM gate slot order (i, f, o, g) -> torch gate rows (i, f, g, o)
    ORDER = [0, 1, 3, 2]

    def q8(x):
        return np.asarray(x, f32).astype(fp8)

    def fold(Wm):
        # Wm [64 out-units, 64 in-units] -> block-diag [128, 128]
        out = np.zeros((128, 128), f32)
        out[0:64, 0:64] = Wm.T
        out[64:128, 64:128] = Wm.T
        return out

    WD0 = np.zeros((128, 4, 2, 128), f32)
    WD1 = np.zeros((128, 4, 2, 128), f32)
    B1T = np.zeros((128, 4), f32)
    for S in range(4):
        X = ORDER[S]
        WD0[:, S, 0, :] = fold(Whh0[X * H:(X + 1) * H])
        we = w_eff[X * H:(X + 1) * H]
        be = b_eff[X * H:(X + 1) * H]
        be_hi = q8(be).astype(f32)
        be_res = (be - be_hi) * 16.0
        WD0[0, S, 1, 0:64] = we
        WD0[64, S, 1, 64:128] = we
        WD0[1, S, 1, 0:64] = be_hi
        WD0[1, S, 1, 64:128] = be_hi
        WD0[2, S, 1, 0:64] = be_res
        WD0[2, S, 1, 64:128] = be_res

        WD1[:, S, 0, :] = fold(Whh1[X * H:(X + 1) * H])
        WD1[:, S, 1, :] = fold(Wih1[X * H:(X + 1) * H])

        B1T[0:64, S] = b1[X * H:(X + 1) * H]
        B1T[64:128, S] = b1[X * H:(X + 1) * H]

    # x-slot fill plane: row1 = 1.0 (bias), row2 = 0.0625 (bias residual),
    # all other non-x rows zero (kills SBUF garbage under zero lhsT rows)
    XPLANE = np.zeros((128, TS * HALF), f32)
    XPLANE[1, :] = 1.0
    XPLANE[2, :] = 0.0625

    HD2 = np.zeros((128, 4), f32)
    HD2[0:64, 0] = inp["mu_W"].astype(f32)[0]
    HD2[0:64, 1] = inp["sigma_W"].astype(f32)[0]
    HD2[64:128, 2] = inp["mu_W"].astype(f32)[0]
    HD2[64:128, 3] = inp["sigma_W"].astype(f32)[0]

    return {
        "WD0": q8(WD0.reshape(128, 1024)),
        "WD1": q8(WD1.reshape(128, 1024)),
        "B1T": B1T,
        "XPLANE": q8(XPLANE),
        "HD2": q8(HD2),
        "IDB": np.eye(128, dtype=f32).astype(ml_dtypes.bfloat16),
        "IDF": np.eye(128, dtype=f32),
        "mu_b": float(inp["mu_b"][0]), "sigma_b": float(inp["sigma_b"][0]),
    }


def _build(mu_b, sigma_b):
    """Build the per-core bass program (SPMD: identical on all cores)."""
    from contextlib import ExitStack
    import concourse.mybir as mybir
    import concourse.tile as tile
    from concourse import bacc

    dt = mybir.dt
    AF = mybir.ActivationFunctionType
    OP = mybir.AluOpType
    DR = mybir.MatmulPerfMode.DoubleRow

    nc = bacc.Bacc()

    # ---- I/O ----------------------------------------------------------
    hist = nc.declare_dram_parameter("hist", [BL, LIN, N], dt.float32, isOutput=False)
    fut = nc.declare_dram_parameter("fut", [BL, LOUT, N], dt.float32, isOutput=False)
    hmask = nc.declare_dram_parameter("hmask", [BL, LIN, N], dt.float32, isOutput=False)
    fmask = nc.declare_dram_parameter("fmask", [BL, LOUT, N], dt.float32, isOutput=False)
    epsin = nc.declare_dram_parameter("epsin", [BL, TS, N], dt.float32, isOutput=False)
    wWD0 = nc.declare_dram_parameter("WD0", [128, 1024], dt.float8e4, isOutput=False)
    wWD1 = nc.declare_dram_parameter("WD1", [128, 1024], dt.float8e4, isOutput=False)
    wB1T = nc.declare_dram_parameter("B1T", [128, 4], dt.float32, isOutput=False)
    wXPL = nc.declare_dram_parameter("XPLANE", [128, TS * HALF], dt.float8e4, isOutput=False)
    wHD2 = nc.declare_dram_parameter("HD2", [128, 4], dt.float8e4, isOutput=False)
    wIDB = nc.declare_dram_parameter("IDB", [128, 128], dt.bfloat16, isOutput=False)
    wIDF = nc.declare_dram_parameter("IDF", [128, 128], dt.float32, isOutput=False)

    o_preds = nc.declare_dram_parameter("preds", [BL, TS, N], dt.float32, isOutput=True)
    o_reals = nc.declare_dram_parameter("reals", [BL, TS, N], dt.float32, isOutput=True)
    o_mus = nc.declare_dram_parameter("musv", [BL, TS, N], dt.float32, isOutput=True)
    o_sigs = nc.declare_dram_parameter("sigmasv", [BL, TS, N], dt.float32, isOutput=True)
    o_mask = nc.declare_dram_parameter("maskv", [BL, TS, N], dt.float32, isOutput=True)

    musig_d = nc.dram_tensor("musig", [4, TS, HALF], dt.float32)
    xs_d = nc.dram_tensor("xsd", [TS, BN], dt.float8e4)

    # HH slot offsets (elements within HH free dim)
    def A_off(t):
        return SLOT + t * SLOT

    def B_off(t):
        return SLOT + t * SLOT + HALF   # B(-1) = 512

    with ExitStack() as ctx:
        tc = ctx.enter_context(tile.TileContext(nc))
        persist = ctx.enter_context(tc.tile_pool(name="persist", bufs=1))
        work = ctx.enter_context(tc.tile_pool(name="work", bufs=2))
        psl0 = ctx.enter_context(tc.tile_pool(name="psl0", bufs=1, space="PSUM"))
        psl1 = ctx.enter_context(tc.tile_pool(name="psl1", bufs=1, space="PSUM"))

        # ---- big tiles / state --------------------------------------------
        HH = persist.tile([128, HHW], dt.float8e4, tag="HH")

        def arows(r0, r1):
            # view of HH rows [r0:r1), A-slots t=0..TS-1: [r1-r0, TS, 512]
            return HH[r0:r1, A_off(0):A_off(TS - 1) + HALF + HALF].rearrange(
                "p (t s) -> p t s", s=SLOT)[:, :, 0:HALF]

        # fill non-x rows of every x slot (bias consts rows 1/2, zeros
        # elsewhere) -- queue-split so transfers overlap pre-pass compute
        xplv = wXPL.rearrange("p (t s) -> p t s", s=HALF)
        nc.sync.dma_start(out=arows(1, 32), in_=xplv[1:32])
        nc.sync.dma_start(out=arows(32, 64), in_=xplv[32:64])
        nc.sync.dma_start(out=arows(65, 96), in_=xplv[65:96])
        nc.scalar.dma_start(out=arows(96, 128), in_=xplv[96:128])

        # ---- weights ------------------------------------------------------
        WD0 = persist.tile([128, 1024], dt.float8e4, tag="WD0")
        WD1 = persist.tile([128, 1024], dt.float8e4, tag="WD1")
        B1T = persist.tile([128, 4], dt.float32, tag="B1T")
        HD2 = persist.tile([128, 4], dt.float8e4, tag="HD2")
        IDB = persist.tile([128, 128], dt.bfloat16, tag="IDB")
        IDF = persist.tile([128, 128], dt.float32, tag="IDF")
        for t_, d_ in [(WD0, wWD0), (WD1, wWD1), (B1T, wB1T),
                       (HD2, wHD2), (IDB, wIDB), (IDF, wIDF)]:
            nc.scalar.dma_start(out=t_[:], in_=d_[:])

        WD0v = WD0.rearrange("p (x g c) -> p x g c", x=4, g=2)
        WD1v = WD1.rearrange("p (x g c) -> p x g c", x=4, g=2)

        g0b = [persist.tile([128, 2048], dt.bfloat16, tag=f"g0b{i}", name=f"g0b{i}")
               for i in range(2)]
        g1b = [persist.tile([128, 2048], dt.bfloat16, tag=f"g1b{i}", name=f"g1b{i}")
               for i in range(2)]
        tc0b = [persist.tile([128, HALF], dt.bfloat16, tag=f"tc0b{i}", name=f"tc0b{i}")
                for i in range(2)]
        tc1b = [persist.tile([128, HALF], dt.bfloat16, tag=f"tc1b{i}", name=f"tc1b{i}")
                for i in range(2)]
        a0 = persist.tile([128, HALF], dt.bfloat16, tag="a0")
        b0 = persist.tile([128, HALF], dt.bfloat16, tag="b0")
        a1 = persist.tile([128, HALF], dt.bfloat16, tag="a1")
        b1t_ = persist.tile([128, HALF], dt.bfloat16, tag="b1t_")
        c0t = persist.tile([128, HALF], dt.bfloat16, tag="c0t")
        c1t = persist.tile([128, HALF], dt.bfloat16, tag="c1t")
        nc.vector.memset(c0t, 0.0)
        nc.vector.memset(c1t, 0.0)

        # pad pair [0:1024): (h1(-1)=0, h0(0) copy) for l1's t=0 DoubleRow;
        # [512:1024) also serves as B(-1) = h0(-1) = 0 for l0's t=0
        nc.vector.memset(HH[:, 0:SLOT], 0.0)

        full_c = []

        # ---- pre-pass: stats, normalize, transpose x ----------------------
        raws = []
        qs = [nc.sync, nc.scalar, nc.sync]
        for c in range(NCHUNK):
            b_, n0 = c // 2, (c % 2) * 128
            raw = work.tile([T, 128], dt.float32, tag="raw", bufs=8, name=f"raw{c}")
            qs[c % 3].dma_start(out=raw[0:LIN, :], in_=hist[b_, :, n0:n0 + 128])
            qs[(c + 1) % 3].dma_start(out=raw[LIN:T, :], in_=fut[b_, :, n0:n0 + 128])
            raws.append(raw)

        MV = persist.tile([128, 16], dt.float32, tag="MV")
        xt8 = persist.tile([TS, BN], dt.float8e4, tag="xt8")
        for c in range(NCHUNK):
            fpt = (psl1 if c % 2 else psl0).tile([128, T], dt.float32,
                                                 tag="l1" if c % 2 else "l0",
                                                 name=f"fpt{c}")
            nc.tensor.transpose(fpt, raws[c], IDF[0:T, 0:T])
            fc = persist.tile([128, T], dt.float32, tag=f"full{c}", name=f"full{c}")
            nc.vector.tensor_copy(fc, fpt)
            st6 = work.tile([128, 6], dt.float32, tag="st6", bufs=2)
            nc.vector.bn_stats(out=st6, in_=fc[:, 0:LIN])
            nc.vector.bn_aggr(out=MV[:, 2 * c:2 * c + 2], in_=st6)
            full_c.append(fc)

        # batched stdev/istd across the 8 chunks: [128, 8] strided views
        MVv = MV.rearrange("p (c k) -> p c k", k=2)
        mean8 = MVv[:, :, 0]
        var8 = MVv[:, :, 1]
        veps = persist.tile([128, 8], dt.float32, tag="veps")
        nc.vector.tensor_scalar(out=veps, in0=var8, scalar1=1e-5,
                                scalar2=None, op0=OP.add)
        y0 = persist.tile([128, 8], dt.float32, tag="y0")
        nc.scalar.activation(y0, veps, AF.Sqrt)
        r0 = work.tile([128, 8], dt.float32, tag="r0")
        nc.vector.reciprocal(r0, y0)
        yy = work.tile([128, 8], dt.float32, tag="yy")
        nc.vector.tensor_tensor(out=yy, in0=y0, in1=y0, op=OP.mult)
        e_ = work.tile([128, 8], dt.float32, tag="e_")
        nc.vector.tensor_tensor(out=e_, in0=veps, in1=yy, op=OP.subtract)
        d_ = work.tile([128, 8], dt.float32, tag="d_")
        nc.vector.scalar_tensor_tensor(out=d_, in0=e_, scalar=0.5, in1=r0,
                                       op0=OP.mult, op1=OP.mult)
        SD = persist.tile([128, 8], dt.float32, tag="SD")
        nc.vector.tensor_tensor(out=SD, in0=y0, in1=d_, op=OP.add)
        ISD = persist.tile([128, 8], dt.float32, tag="ISD")
        nc.vector.reciprocal(ISD, SD)

        for c in range(NCHUNK):
            xn = work.tile([128, TS], dt.bfloat16, tag="xn", bufs=2)
            nc.vector.tensor_scalar(out=xn, in0=full_c[c][:, 0:TS],
                                    scalar1=MVv[:, c, 0:1],
                                    scalar2=ISD[:, c:c + 1],
                                    op0=OP.subtract, op1=OP.mult)
            pt = (psl0 if c % 2 else psl1).tile([TS, 128], dt.bfloat16,
                                                tag="l0" if c % 2 else "l1",
                                                name=f"pt{c}")
            nc.tensor.transpose(pt, xn, IDB)
            nc.vector.tensor_copy(xt8[:, c * 128:(c + 1) * 128], pt)

        # stage xt8 -> DRAM -> HH A-slot rows 0 / 64
        nc.sync.dma_start(out=xs_d[:], in_=xt8[:])
        nc.sync.dma_start(out=arows(0, 1), in_=xs_d[None, :, 0:HALF])
        nc.scalar.dma_start(out=arows(64, 65), in_=xs_d[None, :, HALF:BN])

        # ---- rhs view helper ---------------------------------------------
        def rhs2(off):
            # [128, 2, 512] contiguous DoubleRow rhs starting at element off
            return HH[:, off:off + 2 * HALF].rearrange("p (g s) -> p g s", s=HALF)

        # gate slot offsets within the 2048-wide PSUM: (i, f, o, g)
        GI, GF, GO, GG = 0, HALF, 2 * HALF, 3 * HALF

        # ---- main loop ----------------------------------------------------
        for it in range(TS + 1):
            t = it          # l0 step
            s = it - 1      # l1 step
            if t < TS:
                g0 = g0b[t % 2]
                tc0 = tc0b[t % 2]
                l0ps = psl0.tile([128, 2048], dt.float32, tag="l0", name="l0ps")
                rv0 = rhs2(B_off(t - 1))
                for X in range(4):
                    nc.tensor.matmul(l0ps[:, X * HALF:(X + 1) * HALF],
                                     lhsT=WD0v[:, X], rhs=rv0,
                                     start=True, stop=True, perf_mode=DR)
            if s >= 0:
                g1 = g1b[s % 2]
                tc1 = tc1b[s % 2]
                l1ps = psl1.tile([128, 2048], dt.float32, tag="l1", name="l1ps")
                rv1 = rhs2(A_off(s)) if s >= 1 else rhs2(0)
                # gate emission order f,i,g,o to minimize ACT waits
                for X in (1, 0, 3, 2):
                    nc.tensor.matmul(l1ps[:, X * HALF:(X + 1) * HALF],
                                     lhsT=WD1v[:, X], rhs=rv1,
                                     start=True, stop=True, perf_mode=DR)

            # scalar queue: sig0, tanh0, sf1, si1, TC0, tg1, so1, TC1
            if t < TS:
                nc.scalar.activation(g0[:, 0:3 * HALF], l0ps[:, 0:3 * HALF],
                                     AF.Sigmoid)
                nc.scalar.activation(g0[:, GG:GG + HALF], l0ps[:, GG:GG + HALF],
                                     AF.Tanh)
            if s >= 0:
                nc.scalar.activation(g1[:, GF:GF + HALF], l1ps[:, GF:GF + HALF],
                                     AF.Sigmoid, bias=B1T[:, 1:2])
                nc.scalar.activation(g1[:, GI:GI + HALF], l1ps[:, GI:GI + HALF],
                                     AF.Sigmoid, bias=B1T[:, 0:1])
            if t < TS:
                # DVE: a0 = sf0*c0, b0 = si0*tg0, c0 = a0+b0
                nc.vector.tensor_tensor(out=a0, in0=g0[:, GF:GF + HALF],
                                        in1=c0t, op=OP.mult)
                nc.vector.tensor_tensor(out=b0, in0=g0[:, GI:GI + HALF],
                                        in1=g0[:, GG:GG + HALF], op=OP.mult)
                nc.vector.tensor_tensor(out=c0t, in0=a0, in1=b0, op=OP.add)
                nc.scalar.activation(tc0, c0t, AF.Tanh)
            if s >= 0:
                nc.scalar.activation(g1[:, GG:GG + HALF], l1ps[:, GG:GG + HALF],
                                     AF.Tanh, bias=B1T[:, 3:4])
                nc.scalar.activation(g1[:, GO:GO + HALF], l1ps[:, GO:GO + HALF],
                                     AF.Sigmoid, bias=B1T[:, 2:3])
                nc.vector.tensor_tensor(out=a1, in0=g1[:, GF:GF + HALF],
                                        in1=c1t, op=OP.mult)
            if t < TS:
                # h0(t) -> HH B(t) fp8
                nc.vector.tensor_tensor(out=HH[:, B_off(t):B_off(t) + HALF],
                                        in0=g0[:, GO:GO + HALF], in1=tc0,
                                        op=OP.mult)
                if t == 0:
                    # copy of h0(0) into the pad pair for l1's t=0 DoubleRow
                    nc.vector.tensor_copy(HH[:, HALF:SLOT],
                                          HH[:, B_off(0):B_off(0) + HALF])
            if s >= 0:
                nc.vector.tensor_tensor(out=b1t_, in0=g1[:, GI:GI + HALF],
                                        in1=g1[:, GG:GG + HALF], op=OP.mult)
                nc.vector.tensor_tensor(out=c1t, in0=a1, in1=b1t_, op=OP.add)
                nc.scalar.activation(tc1, c1t, AF.Tanh)
                # h1(s) -> HH A(s+1) fp8
                nc.vector.tensor_tensor(
                    out=HH[:, A_off(s + 1):A_off(s + 1) + HALF],
                    in0=g1[:, GO:GO + HALF], in1=tc1, op=OP.mult)

        # ---- heads tail: mu/sigma for every step from h1 history ----------
        # 4 steps per [4,2048] PSUM tile; the PSUM->SBUF copy is split
        # between the vector and scalar engines so it never stalls the PE
        for k in range(0, TS, 4):
            pool = psl0 if (k // 4) % 2 == 0 else psl1
            tagn = "l0" if (k // 4) % 2 == 0 else "l1"
            hp = pool.tile([4, 2048], dt.float32, tag=tagn, name="hp")
            kk = min(4, TS - k)
            for j in range(kk):
                nc.tensor.matmul(hp[0:4, j * HALF:(j + 1) * HALF],
                                 lhsT=HD2[:, 0:4],
                                 rhs=HH[:, A_off(k + j + 1):A_off(k + j + 1) + HALF],
                                 start=True, stop=True)
            hs = work.tile([4, 2048], dt.float32, tag="hs", bufs=3)
            half_w = (kk * HALF) // 2
            nc.vector.tensor_copy(hs[:, 0:half_w], hp[0:4, 0:half_w])
            nc.scalar.activation(hs[:, half_w:kk * HALF],
                                 hp[0:4, half_w:kk * HALF], AF.Copy)
            qs[(k // 4) % 3].dma_start(
                out=musig_d[:, k:k + kk, :],
                in_=hs[:, 0:kk * HALF].rearrange("p (s b) -> p s b", b=HALF))

        # ---- post-pass ----------------------------------------------------
        c_sigb = persist.tile([128, 1], dt.float32, tag="c_sigb")
        nc.vector.memset(c_sigb, sigma_b)
        c_neg1 = persist.tile([128, 1], dt.float32, tag="c_neg1")
        nc.vector.memset(c_neg1, -1.0)

        for c in range(NCHUNK):
            b_, n0 = c // 2, (c % 2) * 128
            fc = full_c[c]
            mean_s = MVv[:, c, 0:1]
            sd_s = SD[:, c:c + 1]
            hh_, nb = c // 4, c % 4

            # natural-layout input DMAs, spread across queues
            mu_raw = work.tile([TS, 128], dt.float32, tag="mu_raw", bufs=3)
            qs[c % 3].dma_start(out=mu_raw,
                               in_=musig_d[2 * hh_, :, nb * 128:(nb + 1) * 128])
            sg_raw = work.tile([TS, 128], dt.float32, tag="sg_raw", bufs=3)
            qs[(c + 1) % 3].dma_start(out=sg_raw,
                                      in_=musig_d[2 * hh_ + 1, :, nb * 128:(nb + 1) * 128])
            eps_raw = work.tile([TS, 128], dt.float32, tag="eps_raw", bufs=3)
            qs[(c + 2) % 3].dma_start(out=eps_raw, in_=epsin[b_, :, n0:n0 + 128])
            mk_raw = work.tile([TS, 128], dt.float32, tag="mk_raw", bufs=3)
            qs[c % 3].dma_start(out=mk_raw[0:LIN - 1, :],
                                in_=hmask[b_, 1:LIN, n0:n0 + 128])
            qs[(c + 1) % 3].dma_start(out=mk_raw[LIN - 1:TS, :],
                                      in_=fmask[b_, :, n0:n0 + 128])

            def tr_in(raw, nm):
                ps = (psl0 if c % 2 else psl1).tile([128, TS], dt.float32,
                                                    tag="l0" if c % 2 else "l1",
                                                    name=nm + "p")
                nc.tensor.transpose(ps, raw, IDF[0:TS, 0:TS])
                sb = work.tile([128, TS], dt.float32, tag=nm, bufs=2)
                nc.vector.tensor_copy(sb, ps)
                return sb

            mu_t = tr_in(mu_raw, "mu_t")
            sg_t = tr_in(sg_raw, "sg_t")
            eps_c = tr_in(eps_raw, "eps_c")
            mk = tr_in(mk_raw, "mk")

            # sigma = softplus(sg + sigma_b) + 1e-6
            ab_ = work.tile([128, TS], dt.float32, tag="ab_")
            nc.scalar.activation(ab_, sg_t, AF.Abs, bias=c_sigb)
            ex_ = work.tile([128, TS], dt.float32, tag="ex_")
            nc.scalar.activation(ex_, ab_, AF.Exp, scale=c_neg1)
            ln_ = work.tile([128, TS], dt.float32, tag="ln_")
            nc.scalar.activation(ln_, ex_, AF.Ln, bias=1.0)
            rl_ = work.tile([128, TS], dt.float32, tag="rl_")
            nc.vector.tensor_scalar(out=rl_, in0=sg_t, scalar1=sigma_b,
                                    scalar2=0.0, op0=OP.add, op1=OP.max)
            sig = work.tile([128, TS], dt.float32, tag="sig")
            nc.vector.scalar_tensor_tensor(out=sig, in0=ln_, scalar=1e-6, in1=rl_,
                                           op0=OP.add, op1=OP.add)

            # preds = ((mu+mu_b) + sigma*eps)*stdev + means, masked
            m1 = work.tile([128, TS], dt.float32, tag="m1")
            nc.vector.tensor_tensor(out=m1, in0=sig, in1=eps_c, op=OP.mult)
            m2 = work.tile([128, TS], dt.float32, tag="m2")
            nc.vector.scalar_tensor_tensor(out=m2, in0=mu_t, scalar=mu_b, in1=m1,
                                           op0=OP.add, op1=OP.add)
            m3 = work.tile([128, TS], dt.float32, tag="m3")
            nc.vector.tensor_scalar(out=m3, in0=m2, scalar1=sd_s, scalar2=mean_s,
                                    op0=OP.mult, op1=OP.add)
            pr = work.tile([128, TS], dt.float32, tag="pr")
            nc.vector.tensor_tensor(out=pr, in0=m3, in1=mk, op=OP.mult)

            rr = work.tile([128, TS], dt.float32, tag="rr")
            nc.vector.tensor_tensor(out=rr, in0=fc[:, 1:T], in1=mk, op=OP.mult)

            u1_ = work.tile([128, TS], dt.float32, tag="u1_")
            nc.vector.tensor_scalar(out=u1_, in0=mu_t, scalar1=mu_b, scalar2=None,
                                    op0=OP.add)
            u2 = work.tile([128, TS], dt.float32, tag="u2")
            nc.vector.tensor_scalar(out=u2, in0=u1_, scalar1=sd_s, scalar2=mean_s,
                                    op0=OP.mult, op1=OP.add)

            v1_ = work.tile([128, TS], dt.float32, tag="v1_")
            nc.vector.tensor_scalar(out=v1_, in0=sig, scalar1=sd_s, scalar2=mean_s,
                                    op0=OP.mult, op1=OP.add)

            for oi, (src_t, odram) in enumerate(((pr, o_preds), (rr, o_reals),
                                                 (u2, o_mus), (v1_, o_sigs),
                                                 (mk, o_mask))):
                tps = (psl1 if c % 2 else psl0).tile([TS, 128], dt.float32,
                                                     tag="l1" if c % 2 else "l0",
                                                     name="tps")
                nc.tensor.transpose(tps, src_t, IDF)
                osb = work.tile([TS, 128], dt.float32, tag="osb", bufs=4)
                nc.vector.tensor_copy(osb, tps)
                qs[(c + oi) % 3].dma_start(out=odram[b_, :, n0:n0 + 128], in_=osb)

    nc.finalize()
    return nc


def kernel(**inputs):
    import os
    from concourse.bass_utils import run_bass_kernel_spmd

    f32 = np.float32
    packs = _pack_weights(inputs)

    key = "nc"
    if key not in _cache:
        _cache[key] = _build(packs["mu_b"], packs["sigma_b"])
    nc = _cache[key]

    hist = np.ascontiguousarray(np.asarray(inputs["history_data"], f32)[..., 0])
    fut = np.ascontiguousarray(np.asarray(inputs["future_data"], f32)[..., 0])
    hm = np.ascontiguousarray(np.asarray(inputs["history_mask"], f32))
    fm = np.ascontiguousarray(np.asarray(inputs["future_mask"], f32))
    eps = np.ascontiguousarray(np.asarray(inputs["eps"], f32)[..., 0])

    in_maps = []
    for c in range(NCORES):
        b0, b1 = c * BL, (c + 1) * BL
        m = {
            "hist": hist[b0:b1], "fut": fut[b0:b1],
            "hmask": hm[b0:b1], "fmask": fm[b0:b1], "epsin": eps[b0:b1],
        }
        for k in ("WD0", "WD1", "B1T", "XPLANE", "HD2", "IDB", "IDF"):
            m[k] = packs[k]
        in_maps.append(m)

    kres = run_bass_kernel_spmd(nc, in_maps, list(range(NCORES)),
                                trace=bool(os.environ.get("KERNEL_TRACE")))
    _cache["last"] = kres
    res = kres.results

    def gather(name):
        full = np.concatenate([res[c][name] for c in range(NCORES)], axis=0)
        return full.reshape(B, TS, N, 1).astype(f32)

    return (gather("preds"), gather("reals"), gather("musv"),
            gather("sigmasv"), gather("maskv"))


# revision 22
# speedup vs baseline: 1.3285x; 1.1154x over previous
"""DeepAR Trainium2 Bass kernel (sigmoid-native edition).

Strategy (hardcoded from spec nn_DeepAR_90374701843258):
  B=32, LIN=96, LOUT=24, N=256, E=32, H=64, T-1=119 steps, 8 cores.
  Data-parallel over B: 4 batch rows per core -> per-core batch BN=1024.

  Layout: folded tiles: partition p<64 = H-unit p of batch half 0
  (bn 0:512), p>=64 = H-unit p-64 of half 1. Free dim = within-half
  batch column (512 wide).

  PE: gate matmuls fp8(e4m3) DoubleRow, 4 per layer per step.
  Layer0 per gate: groups (Whh0-fold x h0(t-1), [w_eff;bias rows] x
  [x;ones rows]).  Layer1 per gate: groups (Whh1-fold x h1(t-1),
  Wih1-fold x h0(t)); layer1 bias applied in fp32 via the ACT bias
  operand (per-gate [128,1]) -- no bias matmuls.

  Activations: native Sigmoid for i,f,o and Tanh for g / tanh(c) --
  both functions live in the same HW activation table
  (sigmoid_and_others) so there are zero table swaps in the loop.
  Gate PSUM order (i,f,o,g) so layer0 sigmoids are ONE [128,1536] ACT.
  Cell math is pure TENSOR_TENSOR (2x DVE mode): a=sf*c, b=si*tg,
  c'=a+b, h=so*tanh(c') with h written straight to HH as fp8.

  Hidden history big-tile HH (fp8): per step t two 512-wide slots,
  A(t) = x(t) rows {0,64} + ones rows {1,2} (zeros elsewhere, filled
  by queue-split XPLANE DMAs), later overwritten by h1(t-1); B(t) =
  h0(t).  Consecutive slots give contiguous [128,2,512] DoubleRow rhs
  views for both layers; h1 history stays resident so mu/sigma heads
  run post-loop, partition-packed 8 steps per [32,512] PSUM bank.
"""

import numpy as np

B, LIN, LOUT, N, E, H = 32, 96, 24, 256, 32, 64
T = LIN + LOUT
TS = T - 1            # 119
NCORES = 8
BL = B // NCORES      # 4
BN = BL * N           # 1024
HALF = 512
NCHUNK = BN // 128    # 8
SLOT = 1024           # A+B slot pair width per step in HH
HHW = (TS + 2) * SLOT  # HH free width

_cache = {}


def _pack_weights(inp):
    """Host-side weight prep (tiny arrays)."""
    import ml_dtypes
    fp8 = ml_dtypes.float8_e4m3fn
    f32 = np.float32

    Wih0, Whh0 = inp["Wih0"].astype(f32), inp["Whh0"].astype(f32)
    Wih1, Whh1 = inp["Wih1"].astype(f32), inp["Whh1"].astype(f32)
    w_eff = (Wih0 @ inp["embed_W"].astype(f32))[:, 0]
    b_eff = (Wih0 @ inp["embed_b"].astype(f32) + inp["bih0"] + inp["bhh0"]).astype(f32)
    b1 = (inp["bih1"] + inp["bhh1"]).astype(f32)

    # PSUM gate slot order (i, f, o, g) -> torch gate rows (i, f, g, o)
    ORDER = [0, 1, 3, 2]

    def q8(x):
        return np.asarray(x, f32).astype(fp8)

    def fold(Wm):
        # Wm [64 out-units, 64 in-units] -> block-diag [128, 128]
        out = np.zeros((128, 128), f32)
        out[0:64, 0:64] = Wm.T
        out[64:128, 64:128] = Wm.T
        return out

    WD0 = np.zeros((128, 4, 2, 128), f32)
    WD1 = np.zeros((128, 4, 2, 128), f32)
    B1T = np.zeros((128, 4), f32)
    for S in range(4):
        X = ORDER[S]
        WD0[:, S, 0, :] = fold(Whh0[X * H:(X + 1) * H])
        we = w_eff[X * H:(X + 1) * H]
        be = b_eff[X * H:(X + 1) * H]
        be_hi = q8(be).astype(f32)
        be_res = (be - be_hi) * 16.0
        WD0[0, S, 1, 0:64] = we
        WD0[3, S, 1, 64:128] = we
        WD0[1, S, 1, 0:64] = be_hi
        WD0[1, S, 1, 64:128] = be_hi
        WD0[2, S, 1, 0:64] = be_res
        WD0[2, S, 1, 64:128] = be_res

        WD1[:, S, 0, :] = fold(Whh1[X * H:(X + 1) * H])
        WD1[:, S, 1, :] = fold(Wih1[X * H:(X + 1) * H])

        B1T[0:64, S] = b1[X * H:(X + 1) * H]
        B1T[64:128, S] = b1[X * H:(X + 1) * H]

    HD2 = np.zeros((128, 4), f32)
    HD2[0:64, 0] = inp["mu_W"].astype(f32)[0]
    HD2[0:64, 1] = inp["sigma_W"].astype(f32)[0]
    HD2[64:128, 2] = inp["mu_W"].astype(f32)[0]
    HD2[64:128, 3] = inp["sigma_W"].astype(f32)[0]

    return {
        "WD0": q8(WD0.reshape(128, 1024)),
        "WD1": q8(WD1.reshape(128, 1024)),
        "B1T": B1T,
        "HD2": q8(HD2),
        "IDB": np.eye(128, dtype=f32).astype(ml_dtypes.bfloat16),
        "IDF": np.eye(128, dtype=f32),
        "mu_b": float(inp["mu_b"][0]), "sigma_b": float(inp["sigma_b"][0]),
    }


def _build(mu_b, sigma_b):
    """Build the per-core bass program (SPMD: identical on all cores)."""
    from contextlib import ExitStack
    import concourse.mybir as mybir
    import concourse.tile as tile
    from concourse import bacc

    dt = mybir.dt
    AF = mybir.ActivationFunctionType
    OP = mybir.AluOpType
    DR = mybir.MatmulPerfMode.DoubleRow

    nc = bacc.Bacc()

    # ---- I/O ----------------------------------------------------------
    hist = nc.declare_dram_parameter("hist", [BL, LIN, N], dt.float32, isOutput=False)
    fut = nc.declare_dram_parameter("fut", [BL, LOUT, N], dt.float32, isOutput=False)
    hmask = nc.declare_dram_parameter("hmask", [BL, LIN, N], dt.float32, isOutput=False)
    fmask = nc.declare_dram_parameter("fmask", [BL, LOUT, N], dt.float32, isOutput=False)
    epsin = nc.declare_dram_parameter("epsin", [BL, TS, N], dt.float32, isOutput=False)
    wWD0 = nc.declare_dram_parameter("WD0", [128, 1024], dt.float8e4, isOutput=False)
    wWD1 = nc.declare_dram_parameter("WD1", [128, 1024], dt.float8e4, isOutput=False)
    wB1T = nc.declare_dram_parameter("B1T", [128, 4], dt.float32, isOutput=False)
    wHD2 = nc.declare_dram_parameter("HD2", [128, 4], dt.float8e4, isOutput=False)
    wIDB = nc.declare_dram_parameter("IDB", [128, 128], dt.bfloat16, isOutput=False)
    wIDF = nc.declare_dram_parameter("IDF", [128, 128], dt.float32, isOutput=False)

    o_preds = nc.declare_dram_parameter("preds", [BL, TS, N], dt.float32, isOutput=True)
    o_reals = nc.declare_dram_parameter("reals", [BL, TS, N], dt.float32, isOutput=True)
    o_mus = nc.declare_dram_parameter("musv", [BL, TS, N], dt.float32, isOutput=True)
    o_sigs = nc.declare_dram_parameter("sigmasv", [BL, TS, N], dt.float32, isOutput=True)
    o_mask = nc.declare_dram_parameter("maskv", [BL, TS, N], dt.float32, isOutput=True)

    musig_d = nc.dram_tensor("musig", [4, TS, HALF], dt.float32)
    # staging rows for A-slot partitions 0..3: (x half0, ones, res, x half1)
    xs_d = nc.dram_tensor("xsd", [TS, 4 * HALF], dt.float8e4)

    # HH slot offsets (elements within HH free dim)
    def A_off(t):
        return SLOT + t * SLOT

    def B_off(t):
        return SLOT + t * SLOT + HALF   # B(-1) = 512

    with ExitStack() as ctx:
        tc = ctx.enter_context(tile.TileContext(nc))
        persist = ctx.enter_context(tc.tile_pool(name="persist", bufs=1))
        work = ctx.enter_context(tc.tile_pool(name="work", bufs=2))
        psl0 = ctx.enter_context(tc.tile_pool(name="psl0", bufs=1, space="PSUM"))
        psl1 = ctx.enter_context(tc.tile_pool(name="psl1", bufs=1, space="PSUM"))

        # ---- big tiles / state --------------------------------------------
        HH = persist.tile([128, HHW], dt.float8e4, tag="HH")

        def arows(r0, r1):
            # view of HH rows [r0:r1), A-slots t=0..TS-1: [r1-r0, TS, 512]
            return HH[r0:r1, A_off(0):A_off(TS - 1) + HALF + HALF].rearrange(
                "p (t s) -> p t s", s=SLOT)[:, :, 0:HALF]

        # A-slot rows 4:128 only need to be FINITE (zero lhsT rows kill
        # their products, but fp8 NaN garbage would poison the PSUM).
        # Zero them with engine memsets, slot-split across three engines
        # so the ~21us each fully overlap the pre-pass.
        # (full 128 rows -- engine ops need an aligned partition base; the
        # x/const rows 0..3 are DMA-overwritten afterwards)
        zrows = arows(0, 128)
        nc.vector.memset(zrows[:, 0:55, :], 0.0)
        nc.gpsimd.memset(zrows[:, 55:TS, :], 0.0)

        # ---- weights ------------------------------------------------------
        WD0 = persist.tile([128, 1024], dt.float8e4, tag="WD0")
        WD1 = persist.tile([128, 1024], dt.float8e4, tag="WD1")
        B1T = persist.tile([128, 4], dt.float32, tag="B1T")
        HD2 = persist.tile([128, 4], dt.float8e4, tag="HD2")
        IDB = persist.tile([128, 128], dt.bfloat16, tag="IDB")
        IDF = persist.tile([128, 128], dt.float32, tag="IDF")
        for t_, d_ in [(WD0, wWD0), (WD1, wWD1), (B1T, wB1T),
                       (HD2, wHD2), (IDB, wIDB), (IDF, wIDF)]:
            nc.scalar.dma_start(out=t_[:], in_=d_[:])

        WD0v = WD0.rearrange("p (x g c) -> p x g c", x=4, g=2)
        WD1v = WD1.rearrange("p (x g c) -> p x g c", x=4, g=2)

        g0b = [persist.tile([128, 2048], dt.bfloat16, tag=f"g0b{i}", name=f"g0b{i}")
               for i in range(2)]
        g1b = [persist.tile([128, 2048], dt.bfloat16, tag=f"g1b{i}", name=f"g1b{i}")
               for i in range(2)]
        tc0b = [persist.tile([128, HALF], dt.bfloat16, tag=f"tc0b{i}", name=f"tc0b{i}")
                for i in range(2)]
        tc1b = [persist.tile([128, HALF], dt.bfloat16, tag=f"tc1b{i}", name=f"tc1b{i}")
                for i in range(2)]
        a0 = persist.tile([128, HALF], dt.bfloat16, tag="a0")
        b0 = persist.tile([128, HALF], dt.bfloat16, tag="b0")
        a1 = persist.tile([128, HALF], dt.bfloat16, tag="a1")
        b1t_ = persist.tile([128, HALF], dt.bfloat16, tag="b1t_")
        c0t = persist.tile([128, HALF], dt.bfloat16, tag="c0t")
        c1t = persist.tile([128, HALF], dt.bfloat16, tag="c1t")
        nc.vector.memset(c0t, 0.0)
        nc.vector.memset(c1t, 0.0)

        # pad pair [0:1024): (h1(-1)=0, h0(0) copy) for l1's t=0 DoubleRow;
        # [512:1024) also serves as B(-1) = h0(-1) = 0 for l0's t=0
        nc.vector.memset(HH[:, 0:SLOT], 0.0)

        full_c = []

        # ---- pre-pass: stats, normalize, transpose x ----------------------
        raws = []
        qs = [nc.sync, nc.scalar]
        for c in range(NCHUNK):
            b_, n0 = c // 2, (c % 2) * 128
            raw = work.tile([T, 128], dt.float32, tag="raw", bufs=4, name=f"raw{c}")
            nc.sync.dma_start(out=raw[0:LIN, :], in_=hist[b_, :, n0:n0 + 128])
            nc.sync.dma_start(out=raw[LIN:T, :], in_=fut[b_, :, n0:n0 + 128])
            raws.append(raw)

        MV = persist.tile([128, 16], dt.float32, tag="MV")
        # staging layout (x half0, ones, res, x half1) = A-slot rows 0..3
        xt8 = persist.tile([TS, 4 * HALF], dt.float8e4, tag="xt8")
        nc.vector.memset(xt8[:, HALF:2 * HALF], 1.0)
        nc.vector.memset(xt8[:, 2 * HALF:3 * HALF], 0.0625)
        for c in range(NCHUNK):
            fpt = (psl1 if c % 2 else psl0).tile([128, T], dt.float32,
                                                 tag="l1" if c % 2 else "l0",
                                                 name=f"fpt{c}")
            nc.tensor.transpose(fpt, raws[c], IDF[0:T, 0:T])
            fc = persist.tile([128, T], dt.float32, tag=f"full{c}", name=f"full{c}")
            nc.vector.tensor_copy(fc, fpt)
            st6 = work.tile([128, 6], dt.float32, tag="st6", bufs=2)
            nc.vector.bn_stats(out=st6, in_=fc[:, 0:LIN])
            nc.vector.bn_aggr(out=MV[:, 2 * c:2 * c + 2], in_=st6)
            full_c.append(fc)

        # batched stdev/istd across the 8 chunks: [128, 8] strided views
        MVv = MV.rearrange("p (c k) -> p c k", k=2)
        mean8 = MVv[:, :, 0]
        var8 = MVv[:, :, 1]
        veps = persist.tile([128, 8], dt.float32, tag="veps")
        nc.vector.tensor_scalar(out=veps, in0=var8, scalar1=1e-5,
                                scalar2=None, op0=OP.add)
        y0 = persist.tile([128, 8], dt.float32, tag="y0")
        nc.scalar.activation(y0, veps, AF.Sqrt)
        r0 = work.tile([128, 8], dt.float32, tag="r0")
        nc.vector.reciprocal(r0, y0)
        yy = work.tile([128, 8], dt.float32, tag="yy")
        nc.vector.tensor_tensor(out=yy, in0=y0, in1=y0, op=OP.mult)
        e_ = work.tile([128, 8], dt.float32, tag="e_")
        nc.vector.tensor_tensor(out=e_, in0=veps, in1=yy, op=OP.subtract)
        d_ = work.tile([128, 8], dt.float32, tag="d_")
        nc.vector.scalar_tensor_tensor(out=d_, in0=e_, scalar=0.5, in1=r0,
                                       op0=OP.mult, op1=OP.mult)
        SD = persist.tile([128, 8], dt.float32, tag="SD")
        nc.vector.tensor_tensor(out=SD, in0=y0, in1=d_, op=OP.add)
        ISD = persist.tile([128, 8], dt.float32, tag="ISD")
        nc.vector.reciprocal(ISD, SD)

        for c in range(NCHUNK):
            xn = work.tile([128, TS], dt.bfloat16, tag="xn", bufs=2)
            nc.vector.tensor_scalar(out=xn, in0=full_c[c][:, 0:TS],
                                    scalar1=MVv[:, c, 0:1],
                                    scalar2=ISD[:, c:c + 1],
                                    op0=OP.subtract, op1=OP.mult)
            pt = (psl0 if c % 2 else psl1).tile([TS, 128], dt.bfloat16,
                                                tag="l0" if c % 2 else "l1",
                                                name=f"pt{c}")
            nc.tensor.transpose(pt, xn, IDB)
            col = c * 128 if c < 4 else 3 * HALF + (c - 4) * 128
            nc.vector.tensor_copy(xt8[:, col:col + 128], pt)

        # stage xt8 -> DRAM -> HH A-slot rows 0..3 in one DMA
        nc.sync.dma_start(out=xs_d[:], in_=xt8[:])
        nc.sync.dma_start(out=arows(0, 4),
                          in_=xs_d.rearrange("t (r s) -> r t s", s=HALF))

        # ---- rhs view helper ---------------------------------------------
        def rhs2(off):
            # [128, 2, 512] contiguous DoubleRow rhs starting at element off
            return HH[:, off:off + 2 * HALF].rearrange("p (g s) -> p g s", s=HALF)

        # gate slot offsets within the 2048-wide PSUM: (i, f, o, g)
        GI, GF, GO, GG = 0, HALF, 2 * HALF, 3 * HALF

        # ---- main loop ----------------------------------------------------
        for it in range(TS + 1):
            t = it          # l0 step
            s = it - 1      # l1 step
            if t < TS:
                g0 = g0b[t % 2]
                tc0 = tc0b[t % 2]
                l0ps = psl0.tile([128, 2048], dt.float32, tag="l0", name="l0ps")
                rv0 = rhs2(B_off(t - 1))
                for X in range(4):
                    nc.tensor.matmul(l0ps[:, X * HALF:(X + 1) * HALF],
                                     lhsT=WD0v[:, X], rhs=rv0,
                                     start=True, stop=True, perf_mode=DR)
            if s >= 0:
                g1 = g1b[s % 2]
                tc1 = tc1b[s % 2]
                l1ps = psl1.tile([128, 2048], dt.float32, tag="l1", name="l1ps")
                rv1 = rhs2(A_off(s)) if s >= 1 else rhs2(0)
                # gate emission order f,i,g,o to minimize ACT waits
                for X in (1, 0, 3, 2):
                    nc.tensor.matmul(l1ps[:, X * HALF:(X + 1) * HALF],
                                     lhsT=WD1v[:, X], rhs=rv1,
                                     start=True, stop=True, perf_mode=DR)

            # scalar queue: sig0, tanh0, sf1, si1, TC0, tg1, so1, TC1
            if t < TS:
                nc.scalar.activation(g0[:, 0:3 * HALF], l0ps[:, 0:3 * HALF],
                                     AF.Sigmoid)
                nc.scalar.activation(g0[:, GG:GG + HALF], l0ps[:, GG:GG + HALF],
                                     AF.Tanh)
            if s >= 0:
                nc.scalar.activation(g1[:, GF:GF + HALF], l1ps[:, GF:GF + HALF],
                                     AF.Sigmoid, bias=B1T[:, 1:2])
                nc.scalar.activation(g1[:, GI:GI + HALF], l1ps[:, GI:GI + HALF],
                                     AF.Sigmoid, bias=B1T[:, 0:1])
            if t < TS:
                # DVE: a0 = sf0*c0, b0 = si0*tg0, c0 = a0+b0
                nc.vector.tensor_tensor(out=a0, in0=g0[:, GF:GF + HALF],
                                        in1=c0t, op=OP.mult)
                nc.vector.tensor_tensor(out=b0, in0=g0[:, GI:GI + HALF],
                                        in1=g0[:, GG:GG + HALF], op=OP.mult)
                nc.vector.tensor_tensor(out=c0t, in0=a0, in1=b0, op=OP.add)
                nc.scalar.activation(tc0, c0t, AF.Tanh)
            if s >= 0:
                nc.scalar.activation(g1[:, GG:GG + HALF], l1ps[:, GG:GG + HALF],
                                     AF.Tanh, bias=B1T[:, 3:4])
                nc.scalar.activation(g1[:, GO:GO + HALF], l1ps[:, GO:GO + HALF],
                                     AF.Sigmoid, bias=B1T[:, 2:3])
                nc.vector.tensor_tensor(out=a1, in0=g1[:, GF:GF + HALF],
                                        in1=c1t, op=OP.mult)
            if t < TS:
                # h0(t) -> HH B(t) fp8
                nc.vector.tensor_tensor(out=HH[:, B_off(t):B_off(t) + HALF],
                                        in0=g0[:, GO:GO + HALF], in1=tc0,
                                        op=OP.mult)
                if t == 0:
                    # copy of h0(0) into the pad pair for l1's t=0 DoubleRow
                    nc.vector.tensor_copy(HH[:, HALF:SLOT],
                                          HH[:, B_off(0):B_off(0) + HALF])
            if s >= 0:
                nc.vector.tensor_tensor(out=b1t_, in0=g1[:, GI:GI + HALF],
                                        in1=g1[:, GG:GG + HALF], op=OP.mult)
                nc.vector.tensor_tensor(out=c1t, in0=a1, in1=b1t_, op=OP.add)
                nc.scalar.activation(tc1, c1t, AF.Tanh)
                # h1(s) -> HH A(s+1) fp8
                nc.vector.tensor_tensor(
                    out=HH[:, A_off(s + 1):A_off(s + 1) + HALF],
                    in0=g1[:, GO:GO + HALF], in1=tc1, op=OP.mult)

        # ---- heads tail: mu/sigma for every step from h1 history ----------
        # 4 steps per [4,2048] PSUM tile; the PSUM->SBUF copy is split
        # between the vector and scalar engines so it never stalls the PE
        for k in range(0, TS, 4):
            pool = psl0 if (k // 4) % 2 == 0 else psl1
            tagn = "l0" if (k // 4) % 2 == 0 else "l1"
            hp = pool.tile([4, 2048], dt.float32, tag=tagn, name="hp")
            kk = min(4, TS - k)
            for j in range(kk):
                nc.tensor.matmul(hp[0:4, j * HALF:(j + 1) * HALF],
                                 lhsT=HD2[:, 0:4],
                                 rhs=HH[:, A_off(k + j + 1):A_off(k + j + 1) + HALF],
                                 start=True, stop=True)
            hs = work.tile([4, 2048], dt.float32, tag="hs", bufs=2)
            half_w = (kk * HALF) // 2
            nc.vector.tensor_copy(hs[:, 0:half_w], hp[0:4, 0:half_w])
            nc.scalar.activation(hs[:, half_w:kk * HALF],
                                 hp[0:4, half_w:kk * HALF], AF.Copy)
            qs[(k // 4) % 2].dma_start(
                out=musig_d[:, k:k + kk, :],
                in_=hs[:, 0:kk * HALF].rearrange("p (s b) -> p s b", b=HALF))

        # ---- post-pass ----------------------------------------------------

        for c in range(NCHUNK):
            b_, n0 = c // 2, (c % 2) * 128
            fc = full_c[c]
            mean_s = MVv[:, c, 0:1]
            sd_s = SD[:, c:c + 1]
            hh_, nb = c // 4, c % 4

            # natural-layout input DMAs, spread across the two HWDGE queues
            mu_raw = work.tile([TS, 128], dt.float32, tag="mu_raw", bufs=3)
            qs[c % 2].dma_start(out=mu_raw,
                               in_=musig_d[2 * hh_, :, nb * 128:(nb + 1) * 128])
            sg_raw = work.tile([TS, 128], dt.float32, tag="sg_raw", bufs=3)
            qs[(c + 1) % 2].dma_start(out=sg_raw,
                                      in_=musig_d[2 * hh_ + 1, :, nb * 128:(nb + 1) * 128])
            eps_raw = work.tile([TS, 128], dt.float32, tag="eps_raw", bufs=3)
            qs[c % 2].dma_start(out=eps_raw, in_=epsin[b_, :, n0:n0 + 128])
            mk_raw = work.tile([TS, 128], dt.float32, tag="mk_raw", bufs=3)
            qs[(c + 1) % 2].dma_start(out=mk_raw[0:LIN - 1, :],
                                in_=hmask[b_, 1:LIN, n0:n0 + 128])
            qs[c % 2].dma_start(out=mk_raw[LIN - 1:TS, :],
                                      in_=fmask[b_, :, n0:n0 + 128])

            def tr_in(raw, nm):
                ps = (psl0 if c % 2 else psl1).tile([128, TS], dt.float32,
                                                    tag="l0" if c % 2 else "l1",
                                                    name=nm + "p")
                nc.tensor.transpose(ps, raw, IDF[0:TS, 0:TS])
                sb = work.tile([128, TS], dt.float32, tag=nm, bufs=2)
                nc.vector.tensor_copy(sb, ps)
                return sb

            mu_t = tr_in(mu_raw, "mu_t")
            sg_t = tr_in(sg_raw, "sg_t")
            eps_c = tr_in(eps_raw, "eps_c")
            mk = tr_in(mk_raw, "mk")

            # sigma = softplus(z) + 1e-6, z = sg + sigma_b.  -|z| built on
            # the DVE as min(z,-z) so the scalar engine only needs Exp/Ln
            # (one ACT table, no per-chunk table swaps)
            zp_ = work.tile([128, TS], dt.float32, tag="zp_")
            nc.vector.tensor_scalar(out=zp_, in0=sg_t, scalar1=sigma_b,
                                    scalar2=None, op0=OP.add)
            zn_ = work.tile([128, TS], dt.float32, tag="zn_")
            nc.vector.tensor_scalar(out=zn_, in0=sg_t, scalar1=sigma_b,
                                    scalar2=-1.0, op0=OP.add, op1=OP.mult)
            mn_ = work.tile([128, TS], dt.float32, tag="mn_")
            nc.vector.tensor_tensor(out=mn_, in0=zp_, in1=zn_, op=OP.min)
            ex_ = work.tile([128, TS], dt.float32, tag="ex_")
            nc.scalar.activation(ex_, mn_, AF.Exp)
            ln_ = work.tile([128, TS], dt.float32, tag="ln_")
            nc.scalar.activation(ln_, ex_, AF.Ln, bias=1.0)
            rl_ = work.tile([128, TS], dt.float32, tag="rl_")
            nc.vector.tensor_scalar(out=rl_, in0=zp_, scalar1=0.0,
                                    scalar2=None, op0=OP.max)
            sig = work.tile([128, TS], dt.float32, tag="sig")
            nc.vector.scalar_tensor_tensor(out=sig, in0=ln_, scalar=1e-6, in1=rl_,
                                           op0=OP.add, op1=OP.add)

            # preds = ((mu+mu_b) + sigma*eps)*stdev + means, masked
            m1 = work.tile([128, TS], dt.float32, tag="m1")
            nc.vector.tensor_tensor(out=m1, in0=sig, in1=eps_c, op=OP.mult)
            m2 = work.tile([128, TS], dt.float32, tag="m2")
            nc.vector.scalar_tensor_tensor(out=m2, in0=mu_t, scalar=mu_b, in1=m1,
                                           op0=OP.add, op1=OP.add)
            m3 = work.tile([128, TS], dt.float32, tag="m3")
            nc.vector.tensor_scalar(out=m3, in0=m2, scalar1=sd_s, scalar2=mean_s,
                                    op0=OP.mult, op1=OP.add)
            pr = work.tile([128, TS], dt.float32, tag="pr")
            nc.vector.tensor_tensor(out=pr, in0=m3, in1=mk, op=OP.mult)

            rr = work.tile([128, TS], dt.float32, tag="rr")
            nc.vector.tensor_tensor(out=rr, in0=fc[:, 1:T], in1=mk, op=OP.mult)

            u1_ = work.tile([128, TS], dt.float32, tag="u1_")
            nc.vector.tensor_scalar(out=u1_, in0=mu_t, scalar1=mu_b, scalar2=None,
                                    op0=OP.add)
            u2 = work.tile([128, TS], dt.float32, tag="u2")
            nc.vector.tensor_scalar(out=u2, in0=u1_, scalar1=sd_s, scalar2=mean_s,
                                    op0=OP.mult, op1=OP.add)

            v1_ = work.tile([128, TS], dt.float32, tag="v1_")
            nc.vector.tensor_scalar(out=v1_, in0=sig, scalar1=sd_s, scalar2=mean_s,
                                    op0=OP.mult, op1=OP.add)

            for oi, (src_t, odram) in enumerate(((pr, o_preds), (rr, o_reals),
                                                 (u2, o_mus), (v1_, o_sigs),
                                                 (mk, o_mask))):
                tps = (psl1 if c % 2 else psl0).tile([TS, 128], dt.float32,
                                                     tag="l1" if c % 2 else "l0",
                                                     name="tps")
                nc.tensor.transpose(tps, src_t, IDF)
                osb = work.tile([TS, 128], dt.float32, tag="osb", bufs=4)
                nc.vector.tensor_copy(osb, tps)
                qs[(c + oi) % 2].dma_start(out=odram[b_, :, n0:n0 + 128], in_=osb)

    nc.finalize()
    return nc


def kernel(**inputs):
    import os
    from concourse.bass_utils import run_bass_kernel_spmd

    f32 = np.float32
    packs = _pack_weights(inputs)

    key = "nc"
    if key not in _cache:
        _cache[key] = _build(packs["mu_b"], packs["sigma_b"])
    nc = _cache[key]

    hist = np.ascontiguousarray(np.asarray(inputs["history_data"], f32)[..., 0])
    fut = np.ascontiguousarray(np.asarray(inputs["future_data"], f32)[..., 0])
    hm = np.ascontiguousarray(np.asarray(inputs["history_mask"], f32))
    fm = np.ascontiguousarray(np.asarray(inputs["future_mask"], f32))
    eps = np.ascontiguousarray(np.asarray(inputs["eps"], f32)[..., 0])

    in_maps = []
    for c in range(NCORES):
        b0, b1 = c * BL, (c + 1) * BL
        m = {
            "hist": hist[b0:b1], "fut": fut[b0:b1],
            "hmask": hm[b0:b1], "fmask": fm[b0:b1], "epsin": eps[b0:b1],
        }
        for k in ("WD0", "WD1", "B1T", "HD2", "IDB", "IDF"):
            m[k] = packs[k]
        in_maps.append(m)

    kres = run_bass_kernel_spmd(nc, in_maps, list(range(NCORES)),
                                trace=bool(os.environ.get("KERNEL_TRACE")))
    _cache["last"] = kres
    res = kres.results

    def gather(name):
        full = np.concatenate([res[c][name] for c in range(NCORES)], axis=0)
        return full.reshape(B, TS, N, 1).astype(f32)

    return (gather("preds"), gather("reals"), gather("musv"),
            gather("sigmasv"), gather("maskv"))
